# revision 7
# baseline (speedup 1.0000x reference)
"""CausalDiT block on 8 TRN2 NeuronCores.

Sharding: frame-strided sequence shard — core r owns rows [512f+64r, 512f+64r+64)
for every frame f (perfect causal balance, SPMD-uniform suffix structure).
Self/cross-attention QKV+O computed on own rows with full (bf16) weights;
K/V (pre-transposed K) AllGathered; FFN is tensor-parallel on the hidden dim
with AllGather(inp^T) + ReduceScatter(partial y).

All matmuls run in bf16 (weights cast host-side), accumulation fp32 in PSUM;
norms/softmax/residual chains in fp32. RoPE is applied split-half after a
host-side even/odd permutation of wq/wk output columns (dot-product invariant).
"""
import numpy as np
import ml_dtypes

import concourse.bass as bass
import concourse.mybir as mybir
import concourse.tile as tile
from concourse import bacc
from concourse.bass_utils import run_bass_kernel_spmd
from concourse.masks import make_identity

BF = mybir.dt.bfloat16
F32 = mybir.dt.float32
NPBF = ml_dtypes.bfloat16

NCORES = 8
T, D, H, HD, F, S, FF, TC = 3072, 1536, 12, 128, 6, 512, 8960, 512
FFS = FF // NCORES            # 1120 hidden per core
FFP = 1152                    # padded to 9*128
NFFC = 9                      # ff chunks (8 full + one 96)
NB = 3                        # local 128-row blocks (384 rows/core)
NCH = D // 128                # 12 chunks of the model dim
EPS = 1e-6
SCALE = float(1.0 / np.sqrt(HD))
KVSEG = D * 384               # elements of one packed AG segment (kT or v)
CASEG = D * 64

# vecs rows
VR_SH_MSA, VR_SC_MSA, VR_G_MSA = 0, 6, 12
VR_SH_MLP, VR_SC_MLP, VR_G_MLP = 18, 24, 30
VR_N3W, VR_N3B, VR_NQ, VR_NK, VR_CNQ, VR_CNK = 36, 37, 38, 39, 40, 41
VR_BQ, VR_BK, VR_BV, VR_BO, VR_CBQ, VR_CBK, VR_CBV, VR_CBO, VR_B2 = range(42, 51)

_rope_perm_head = np.concatenate([np.arange(0, HD, 2), np.arange(1, HD, 2)])
ROPE_PERM = np.concatenate([h * HD + _rope_perm_head for h in range(H)])


def _rows_of_core(r):
    return np.concatenate([512 * f + 64 * r + np.arange(64) for f in range(F)])


def _bcast_rows(handle, row, nparts, ncols=D):
    """DRAM AP replicating `row` of a [R, ncols] tensor across nparts partitions."""
    return bass.AP(tensor=handle, offset=row * ncols, ap=[[0, nparts], [1, ncols]])


def _build():
    nc = bacc.Bacc("TRN2", target_bir_lowering=False, debug=False, num_devices=NCORES)

    di = {}
    def inp(name, shape, dt):
        di[name] = nc.dram_tensor(name, shape, dt, kind="ExternalInput")
        return di[name]

    xs = inp("xs", [384, D], F32)
    cs = inp("cs", [384, 128], F32)            # [cos | sin] per own row
    for w in ["wq", "wk", "wv", "wo", "cq", "ck", "cv", "cw"]:
        inp(w, [D, D], BF)
    ctxT = inp("ctxT", [D, 64], BF)
    w1 = inp("w1", [D, FFS], BF)
    w2 = inp("w2", [FFP, D], BF)
    b1 = inp("b1", [128, NFFC], F32)
    vecs = inp("vecs", [51, D], F32)
    out = nc.dram_tensor("y", [384, D], F32, kind="ExternalOutput")

    def wview(name):
        return di[name].ap().rearrange("(o p) f -> p o f", p=128)

    with tile.TileContext(nc) as tc:
        import contextlib
        ctx = contextlib.ExitStack()
        with ctx:
            # ---- persistent pools ----
            dram = ctx.enter_context(tc.tile_pool(name="dram", bufs=1, space="DRAM"))
            singles = ctx.enter_context(tc.tile_pool(name="singles", bufs=1))
            xp = ctx.enter_context(tc.tile_pool(name="xp", bufs=1))
            scr = ctx.enter_context(tc.tile_pool(name="scr", bufs=2))
            statp = ctx.enter_context(tc.tile_pool(name="statp", bufs=2))
            replp = ctx.enter_context(tc.tile_pool(name="replp", bufs=2))
            actbf = ctx.enter_context(tc.tile_pool(name="actbf", bufs=2))
            tpTp = ctx.enter_context(tc.tile_pool(name="tpTp", bufs=1))
            qTp = ctx.enter_context(tc.tile_pool(name="qTp", bufs=1))
            aoTp = ctx.enter_context(tc.tile_pool(name="aoTp", bufs=1))
            pbfp = ctx.enter_context(tc.tile_pool(name="pbfp", bufs=2))
            rrep = ctx.enter_context(tc.tile_pool(name="rrep", bufs=2))
            # psum pools
            ps_pp = ctx.enter_context(tc.tile_pool(name="ps_pp", bufs=2, space="PSUM"))
            ps_sp = ctx.enter_context(tc.tile_pool(name="ps_sp", bufs=2, space="PSUM"))
            ps_op = ctx.enter_context(tc.tile_pool(name="ps_op", bufs=2, space="PSUM"))
            ps_mp = ctx.enter_context(tc.tile_pool(name="ps_mp", bufs=1, space="PSUM"))
            ps_tp = ctx.enter_context(tc.tile_pool(name="ps_tp", bufs=1, space="PSUM"))

            # ---- constants ----
            idon = singles.tile([128, 256], BF)          # [:,0:128] identity, [:,128:256] ones
            make_identity(nc, idon[:, 0:128])
            nc.vector.memset(idon[:, 128:256], 1.0)
            ones_mat = idon[:, 128:256]                  # [128,128] of ones
            cs_sb = singles.tile([128, NB, 128], F32)    # [p, b, cos|sin]
            nc.sync.dma_start(cs_sb, cs.ap().rearrange("(b p) d -> p b d", p=128))
            b1_sb = singles.tile([128, NFFC], F32)
            nc.sync.dma_start(b1_sb, b1.ap())
            eps_sb = singles.tile([128, 1], F32)
            nc.vector.memset(eps_sb, EPS)
            x_sb = xp.tile([128, NB, D], F32)
            nc.sync.dma_start(x_sb, xs.ap().rearrange("(b p) f -> p b f", p=128))

            # ---- DRAM bounce buffers ----
            kv_in = dram.tile([2 * KVSEG], BF)
            kv_out = dram.tile([NCORES * 2 * KVSEG], BF)
            ca_in = dram.tile([2 * CASEG], BF)
            ca_out = dram.tile([NCORES * 2 * CASEG], BF)
            i2_in = dram.tile([KVSEG], BF)
            i2_out = dram.tile([NCORES * KVSEG], BF)
            y2_in = dram.tile([T * D], F32)
            y2_out = dram.tile([384 * D], F32)

            # ================= helpers =================
            def repl(rows64=None, row=None, nparts=128):
                """Replicated [128, D] f32 tile from vecs rows (DMA broadcast).
                rows64: list of (part_off, nparts, row) half-fills."""
                t = replp.tile([128, D], F32, tag="repl")
                if rows64 is not None:
                    for off, n, rw in rows64:
                        nc.sync.dma_start(t[off:off + n, :], _bcast_rows(vecs, rw, n))
                else:
                    nc.sync.dma_start(t[:nparts, :], _bcast_rows(vecs, row, nparts))
                return t

            def frame_tiles(b, base_row):
                """(1+sc)/sh/g style tile for block b: frames 2b (rows 0-63), 2b+1."""
                return repl(rows64=[(0, 64, base_row + 2 * b), (64, 64, base_row + 2 * b + 1)])

            def layer_norm(src_ap, c_tile, s_tile, out_bf):
                """out_bf (bf16 [128,D]) = LN(src) * c_tile + s_tile."""
                st = statp.tile([128, 24], F32, tag="stat")
                sv = src_ap.rearrange("p (n d) -> p n d", d=512)
                for i in range(3):
                    nc.vector.bn_stats(st[:, 6 * i:6 * i + 6], sv[:, i, :])
                nc.vector.bn_aggr(st[:, 18:20], st[:, 0:18].rearrange("p (n d) -> p n d", d=6))
                nc.scalar.activation(st[:, 20:21], st[:, 19:20],
                                     mybir.ActivationFunctionType.Sqrt, bias=eps_sb)
                nc.vector.reciprocal(st[:, 20:21], st[:, 20:21])
                t1 = scr.tile([128, D], F32, tag="scr")
                nc.vector.tensor_scalar(out=t1, in0=src_ap,
                                        scalar1=st[:, 18:19], scalar2=st[:, 20:21],
                                        op0=mybir.AluOpType.subtract,
                                        op1=mybir.AluOpType.mult)
                t2 = scr.tile([128, D], F32, tag="scr")
                nc.vector.tensor_tensor(out=t2, in0=t1, in1=c_tile, op=mybir.AluOpType.mult)
                nc.vector.tensor_tensor(out=out_bf, in0=t2, in1=s_tile, op=mybir.AluOpType.add)

            def transpose_into(src_bf, dst, coff, nch=NCH, rows=128):
                """PE-transpose src_bf [rows, nch*128] into dst[:, c, coff:coff+rows]."""
                for c in range(nch):
                    pt = ps_tp.tile([128, 128], BF, tag="tr")
                    nc.tensor.transpose(pt[:, :rows], src_bf[:rows, c * 128:(c + 1) * 128],
                                        idon[:rows, 0:rows])
                    nc.vector.tensor_copy(out=dst[:, c, coff:coff + rows], in_=pt[:, :rows])

            def project(wname, brow, lhsT_sb, evict, m=128, nb=NB):
                """psum[b] = lhsT[b].T @ W ; evict(b, ob, psum[:m,:512], bias_tile)."""
                wv = wview(wname)
                bias_t = repl(row=brow, nparts=m)
                for ob in range(3):
                    wt = bigw.tile([128, NCH, 512], BF, tag="bigw")
                    nc.sync.dma_start(wt, wv[:, :, ob * 512:(ob + 1) * 512])
                    for b in range(nb):
                        ps = ps_pp.tile([128, 512], F32, tag="pp")
                        for c in range(NCH):
                            nc.tensor.matmul(ps[:m, :], lhsT=lhsT_sb[:, c, b * 128:b * 128 + m],
                                             rhs=wt[:, c, :], start=(c == 0),
                                             stop=(c == NCH - 1))
                        evict(b, ob, ps, bias_t)

            def rms_norm_rope(qf_tiles, nrow, dst_T, rope):
                """qf_tiles: 3 blocks [128,D] f32 -> rms(nq)+optional rope -> bf16 -> dst_T."""
                nrm = repl(row=nrow)
                for b in range(NB):
                    st = statp.tile([128, 24], F32, tag="stat")
                    sq = scr.tile([128, D], F32, tag="scr")
                    nc.scalar.activation(sq, qf_tiles[b], mybir.ActivationFunctionType.Square,
                                         accum_out=st[:, 0:1])
                    nc.scalar.activation(st[:, 1:2], st[:, 0:1],
                                         mybir.ActivationFunctionType.Sqrt,
                                         bias=eps_sb, scale=1.0 / D)
                    nc.vector.reciprocal(st[:, 1:2], st[:, 1:2])
                    qbf = actbf.tile([128, D], BF, tag="abf")
                    if not rope:
                        nc.vector.scalar_tensor_tensor(out=qbf, in0=qf_tiles[b],
                                                       scalar=st[:, 1:2], in1=nrm,
                                                       op0=mybir.AluOpType.mult,
                                                       op1=mybir.AluOpType.mult)
                    else:
                        qn = scr.tile([128, D], F32, tag="scr")
                        nc.vector.scalar_tensor_tensor(out=qn, in0=qf_tiles[b],
                                                       scalar=st[:, 1:2], in1=nrm,
                                                       op0=mybir.AluOpType.mult,
                                                       op1=mybir.AluOpType.mult)
                        qv = qn.rearrange("p (h two d) -> p h two d", two=2, d=64)
                        ov = qbf.rearrange("p (h two d) -> p h two d", two=2, d=64)
                        cb = cs_sb[:, b:b + 1, 0:64].to_broadcast((128, H, 64))
                        sb_ = cs_sb[:, b:b + 1, 64:128].to_broadcast((128, H, 64))
                        tt = scr.tile([128, D], F32, tag="scr")
                        t1 = tt[:, 0:768].rearrange("p (h d) -> p h d", d=64)
                        t2 = tt[:, 768:1536].rearrange("p (h d) -> p h d", d=64)
                        top, bot = qv[:, :, 0, :], qv[:, :, 1, :]
                        nc.vector.tensor_tensor(out=t1, in0=top, in1=cb, op=mybir.AluOpType.mult)
                        nc.vector.tensor_tensor(out=t2, in0=bot, in1=sb_, op=mybir.AluOpType.mult)
                        nc.vector.tensor_tensor(out=ov[:, :, 0, :], in0=t1, in1=t2,
                                                op=mybir.AluOpType.subtract)
                        nc.vector.tensor_tensor(out=t1, in0=top, in1=sb_, op=mybir.AluOpType.mult)
                        nc.vector.tensor_tensor(out=t2, in0=bot, in1=cb, op=mybir.AluOpType.mult)
                        nc.vector.tensor_tensor(out=ov[:, :, 1, :], in0=t1, in1=t2,
                                                op=mybir.AluOpType.add)
                    transpose_into(qbf, dst_T, b * 128)

            def attention(qT, kt_of, vh_of, kseq, dst_oT):
                """Generic S^T attention.
                kt_of(h) -> [128, nkt*128] lhsT source tile (head dims x keys per tile)
                vh_of(h) -> [128, nkt*128]-style v tile accessor
                kseq: list of (ktile_idx, qoff) in processing order."""
                for h in range(H):
                    kt, vh = kt_of(h), vh_of(h)
                    ps_o = ps_op.tile([128, 384], F32, tag="po")
                    ps_m = ps_mp.tile([128, 384], F32, tag="pm")
                    for i, (ki, qo) in enumerate(kseq):
                        n = 384 - qo
                        ps_s = ps_sp.tile([128, 384], F32, tag="ps")
                        nc.tensor.matmul(ps_s[:, 0:n], lhsT=kt[:, ki * 128:(ki + 1) * 128],
                                         rhs=qT[:, h, qo:384], start=True, stop=True)
                        pbt = pbfp.tile([128, 384], BF, tag="pbf")
                        nc.scalar.activation(pbt[:, 0:n], ps_s[:, 0:n],
                                             mybir.ActivationFunctionType.Exp, scale=SCALE)
                        last = i == len(kseq) - 1
                        nc.tensor.matmul(ps_o[:, qo:384], lhsT=vh[:, ki * 128:(ki + 1) * 128],
                                         rhs=pbt[:, 0:n], start=(i == 0), stop=last,
                                         skip_group_check=True)
                        nc.tensor.matmul(ps_m[:, qo:384], lhsT=ones_mat,
                                         rhs=pbt[:, 0:n], start=(i == 0), stop=last,
                                         skip_group_check=True)
                    rc = rrep.tile([128, 384], F32, tag="rr")
                    nc.vector.reciprocal(rc, ps_m)
                    nc.vector.tensor_tensor(out=dst_oT[:, h, :], in0=ps_o, in1=rc,
                                            op=mybir.AluOpType.mult)

            # ============================================================
            # Phase 1+2: self-attention & cross-attention
            # ============================================================
            with tc.tile_pool(name="bigw", bufs=2) as bigw, \
                 tc.tile_pool(name="qfp", bufs=3) as qfp, \
                 tc.tile_pool(name="vsb", bufs=1) as vsbp, \
                 tc.tile_pool(name="midp", bufs=2) as midp:

                # ---- LN1 + modulation -> inp (bf16, transposed) ----
                inpT = tpTp.tile([128, NCH, 384], BF, tag="tpT")
                for b in range(NB):
                    c_t = frame_tiles(b, VR_SC_MSA)
                    s_t = frame_tiles(b, VR_SH_MSA)
                    ibf = actbf.tile([128, D], BF, tag="abf")
                    layer_norm(x_sb[:, b, :], c_t, s_t, ibf)
                    transpose_into(ibf, inpT, b * 128)

                # ---- QKV projections ----
                qf = [qfp.tile([128, D], F32, tag="qf", name=f"qf{i}") for i in range(NB)]
                def ev_f32(tiles):
                    def e(b, ob, ps, bias_t):
                        sl = slice(ob * 512, (ob + 1) * 512)
                        nc.vector.tensor_tensor(out=tiles[b][:, sl], in0=ps,
                                                in1=bias_t[:, sl], op=mybir.AluOpType.add)
                    return e
                qT = qTp.tile([128, NCH, 384], BF, tag="qT")
                project("wq", VR_BQ, inpT, ev_f32(qf))
                rms_norm_rope(qf, VR_NQ, qT, rope=True)

                kf = [qfp.tile([128, D], F32, tag="qf", name=f"kf{i}") for i in range(NB)]
                project("wk", VR_BK, inpT, ev_f32(kf))
                kT_l = tpTp.tile([128, NCH, 384], BF, tag="kTl")
                rms_norm_rope(kf, VR_NK, kT_l, rope=True)

                v_sb = vsbp.tile([128, NB, D], BF, tag="vsb")
                def ev_v(b, ob, ps, bias_t):
                    sl = slice(ob * 512, (ob + 1) * 512)
                    nc.vector.tensor_tensor(out=v_sb[:, b, sl], in0=ps,
                                            in1=bias_t[:, sl], op=mybir.AluOpType.add)
                project("wv", VR_BV, inpT, ev_v)

                # ---- pack + AllGather kT|v ----
                kT_seg = kv_in[0:KVSEG].rearrange("(a b) -> a b", a=D)
                v_seg = kv_in[KVSEG:2 * KVSEG].rearrange("(a b) -> a b", a=384)
                nc.sync.dma_start(kT_seg.rearrange("(o p) f -> p o f", p=128), kT_l)
                nc.sync.dma_start(v_seg.rearrange("(o p) f -> p o f", p=128), v_sb)
                nc.gpsimd.collective_compute(
                    "AllGather", mybir.AluOpType.bypass,
                    replica_groups=[list(range(NCORES))],
                    ins=[kv_in[:].opt()], outs=[kv_out[:].opt()])

                # ---- SA attention ----
                SA_SEQ = [(f * 4 + j, 64 * f) for f in range(F) for j in range(4)]

                def sa_kt2(h):
                    kt = midp.tile([128, F * 512], BF, tag="mid")
                    ktv = kt.rearrange("p (f d) -> p f d", d=512)
                    for rr in range(NCORES):
                        seg = kv_out[rr * 2 * KVSEG: rr * 2 * KVSEG + KVSEG] \
                            .rearrange("(a b) -> a b", a=D)
                        nc.sync.dma_start(
                            ktv[:, :, rr * 64:(rr + 1) * 64],
                            seg[h * 128:(h + 1) * 128, :].rearrange("p (f d) -> p f d", d=64))
                    return kt

                def sa_vh(h):
                    vh = midp.tile([128, F * 512], BF, tag="mid")
                    vhv = vh.rearrange("p (f d) -> p f d", d=512)
                    for rr in range(NCORES):
                        seg = kv_out[rr * 2 * KVSEG + KVSEG: (rr + 1) * 2 * KVSEG] \
                            .rearrange("(a b) -> a b", a=384)
                        nc.sync.dma_start(
                            vhv[64 * (rr % 2):64 * (rr % 2) + 64, :,
                                (rr // 2) * 128:(rr // 2) * 128 + 128],
                            seg.rearrange("(f p) c -> p f c", p=64)[:, :, h * 128:(h + 1) * 128])
                    return vh

                attn_oT = aoTp.tile([128, NCH, 384], BF, tag="aoT")
                attention(qT, sa_kt2, sa_vh, SA_SEQ, attn_oT)

                # ---- output projection + gated residual ----
                ysa = [qfp.tile([128, D], F32, tag="qf", name=f"ysa{i}") for i in range(NB)]
                project("wo", VR_BO, attn_oT, ev_f32(ysa))
                for b in range(NB):
                    g_t = frame_tiles(b, VR_G_MSA)
                    t1 = scr.tile([128, D], F32, tag="scr")
                    nc.vector.tensor_tensor(out=t1, in0=ysa[b], in1=g_t, op=mybir.AluOpType.mult)
                    nc.vector.tensor_tensor(out=x_sb[:, b, :], in0=x_sb[:, b, :], in1=t1,
                                            op=mybir.AluOpType.add)

                # ============ cross attention ============
                n3w = repl(row=VR_N3W)
                n3b = repl(row=VR_N3B)
                xnT = tpTp.tile([128, NCH, 384], BF, tag="tpT")
                for b in range(NB):
                    xbf = actbf.tile([128, D], BF, tag="abf")
                    layer_norm(x_sb[:, b, :], n3w, n3b, xbf)
                    transpose_into(xbf, xnT, b * 128)

                qcf = [qfp.tile([128, D], F32, tag="qf", name=f"qcf{i}") for i in range(NB)]
                project("cq", VR_CBQ, xnT, ev_f32(qcf))
                qcT = qTp.tile([128, NCH, 384], BF, tag="qT")
                rms_norm_rope(qcf, VR_CNQ, qcT, rope=False)

                # context k/v on own 64 context rows
                ctxT_sb = singles.tile([128, NCH, 64], BF)
                nc.sync.dma_start(ctxT_sb, ctxT.ap().rearrange("(o p) f -> p o f", p=128))
                kc_f = qfp.tile([128, D], F32, tag="qf")
                def ev_kc(b, ob, ps, bias_t):
                    sl = slice(ob * 512, (ob + 1) * 512)
                    nc.vector.tensor_tensor(out=kc_f[0:64, sl], in0=ps[:64, :],
                                            in1=bias_t[:64, sl], op=mybir.AluOpType.add)
                project("ck", VR_CBK, ctxT_sb, ev_kc, m=64, nb=1)
                vc_bf = actbf.tile([128, D], BF, tag="abf")
                def ev_vc(b, ob, ps, bias_t):
                    sl = slice(ob * 512, (ob + 1) * 512)
                    nc.vector.tensor_tensor(out=vc_bf[0:64, sl], in0=ps[:64, :],
                                            in1=bias_t[:64, sl], op=mybir.AluOpType.add)
                project("cv", VR_CBV, ctxT_sb, ev_vc, m=64, nb=1)

                # rms norm kc (64 rows)
                cnk = repl(row=VR_CNK, nparts=64)
                stc = statp.tile([128, 24], F32, tag="stat")
                sqc = scr.tile([128, D], F32, tag="scr")
                nc.scalar.activation(sqc[:64, :], kc_f[:64, :],
                                     mybir.ActivationFunctionType.Square,
                                     accum_out=stc[:64, 0:1])
                nc.scalar.activation(stc[:64, 1:2], stc[:64, 0:1],
                                     mybir.ActivationFunctionType.Sqrt, bias=eps_sb[:64],
                                     scale=1.0 / D)
                nc.vector.reciprocal(stc[:64, 1:2], stc[:64, 1:2])
                kc_bf = actbf.tile([128, D], BF, tag="abf")
                nc.vector.scalar_tensor_tensor(out=kc_bf[:64, :], in0=kc_f[:64, :],
                                               scalar=stc[:64, 1:2], in1=cnk[:64, :],
                                               op0=mybir.AluOpType.mult,
                                               op1=mybir.AluOpType.mult)
                kcT_l = tpTp.tile([128, NCH, 64], BF, tag="kcTl")
                transpose_into(kc_bf, kcT_l, 0, rows=64)

                # pack + AllGather ca kT|v
                kcT_seg = ca_in[0:CASEG].rearrange("(a b) -> a b", a=D)
                vc_seg = ca_in[CASEG:2 * CASEG].rearrange("(a b) -> a b", a=64)
                nc.sync.dma_start(kcT_seg.rearrange("(o p) f -> p o f", p=128), kcT_l)
                nc.sync.dma_start(vc_seg, vc_bf[:64, :])
                nc.gpsimd.collective_compute(
                    "AllGather", mybir.AluOpType.bypass,
                    replica_groups=[list(range(NCORES))],
                    ins=[ca_in[:].opt()], outs=[ca_out[:].opt()])

                CA_SEQ = [(t, 0) for t in range(4)]

                def ca_kt(h):
                    kt = midp.tile([128, F * 512], BF, tag="mid")
                    for rr in range(NCORES):
                        seg = ca_out[rr * 2 * CASEG: rr * 2 * CASEG + CASEG] \
                            .rearrange("(a b) -> a b", a=D)
                        nc.sync.dma_start(kt[:, rr * 64:(rr + 1) * 64],
                                          seg[h * 128:(h + 1) * 128, :])
                    return kt

                def ca_vh(h):
                    vh = midp.tile([128, F * 512], BF, tag="mid")
                    for rr in range(NCORES):
                        seg = ca_out[rr * 2 * CASEG + CASEG: (rr + 1) * 2 * CASEG] \
                            .rearrange("(a b) -> a b", a=64)
                        nc.sync.dma_start(
                            vh[64 * (rr % 2):64 * (rr % 2) + 64,
                               (rr // 2) * 128:(rr // 2) * 128 + 128],
                            seg[:, h * 128:(h + 1) * 128])
                    return vh

                attn2_oT = aoTp.tile([128, NCH, 384], BF, tag="aoT")
                attention(qcT, ca_kt, ca_vh, CA_SEQ, attn2_oT)

                # output projection, residual (no gate)
                def ev_addx(b, ob, ps, bias_t):
                    sl = slice(ob * 512, (ob + 1) * 512)
                    t9 = scr.tile([128, 512], F32, tag="scr512")
                    nc.vector.tensor_tensor(out=t9, in0=ps, in1=bias_t[:, sl],
                                            op=mybir.AluOpType.add)
                    nc.vector.tensor_tensor(out=x_sb[:, b, sl], in0=x_sb[:, b, sl],
                                            in1=t9, op=mybir.AluOpType.add)
                project("cw", VR_CBO, attn2_oT, ev_addx)

            # ============================================================
            # Phase 3: FFN (tensor-parallel on hidden dim)
            # ============================================================
            with tc.tile_pool(name="w1p", bufs=1) as w1p, \
                 tc.tile_pool(name="w2p", bufs=1) as w2p, \
                 tc.tile_pool(name="i2p", bufs=2) as i2p, \
                 tc.tile_pool(name="hTp", bufs=2) as hTp, \
                 tc.tile_pool(name="y2e", bufs=2) as y2e:

                inp2T = tpTp.tile([128, NCH, 384], BF, tag="tpT")
                for b in range(NB):
                    c_t = frame_tiles(b, VR_SC_MLP)
                    s_t = frame_tiles(b, VR_SH_MLP)
                    ibf = actbf.tile([128, D], BF, tag="abf")
                    layer_norm(x_sb[:, b, :], c_t, s_t, ibf)
                    transpose_into(ibf, inp2T, b * 128)
                i2_seg = i2_in[:].rearrange("(a b) -> a b", a=D)
                nc.sync.dma_start(i2_seg.rearrange("(o p) f -> p o f", p=128), inp2T)
                nc.gpsimd.collective_compute(
                    "AllGather", mybir.AluOpType.bypass,
                    replica_groups=[list(range(NCORES))],
                    ins=[i2_in[:].opt()], outs=[i2_out[:].opt()])

                w1_sb = w1p.tile([128, NCH, FFS], BF, tag="w1")
                nc.sync.dma_start(w1_sb, w1.ap().rearrange("(o p) f -> p o f", p=128))
                w2_sb = w2p.tile([128, NFFC, D], BF, tag="w2")
                b2_t = repl(row=VR_B2)
                nc.sync.dma_start(w2_sb, w2.ap().rearrange("(o p) f -> p o f", p=128))
                y2b = y2_in[:].rearrange("(a b) -> a b", a=T)

                for rr in range(NCORES):
                    i2t = i2p.tile([128, NCH, 384], BF, tag="i2")
                    seg = i2_out[rr * KVSEG:(rr + 1) * KVSEG].rearrange("(a b) -> a b", a=D)
                    nc.sync.dma_start(i2t, seg.rearrange("(o p) f -> p o f", p=128))
                    hT = hTp.tile([128, NFFC, 384], BF, tag="hT")
                    for fc in range(NFFC):
                        rows = 96 if fc == NFFC - 1 else 128
                        ps = ps_pp.tile([128, 512], F32, tag="pp")
                        for c in range(NCH):
                            nc.tensor.matmul(ps[:rows, 0:384],
                                             lhsT=w1_sb[:, c, fc * 128:fc * 128 + rows],
                                             rhs=i2t[:, c, :], start=(c == 0),
                                             stop=(c == NCH - 1))
                        nc.scalar.activation(hT[:rows, fc, :], ps[:rows, 0:384],
                                             mybir.ActivationFunctionType.Gelu_apprx_tanh,
                                             bias=b1_sb[:rows, fc:fc + 1])
                    for o in range(NB):
                        for ob in range(3):
                            ps = ps_pp.tile([128, 512], F32, tag="pp")
                            for fc in range(NFFC):
                                rows = 96 if fc == NFFC - 1 else 128
                                nc.tensor.matmul(
                                    ps, lhsT=hT[:rows, fc, o * 128:(o + 1) * 128],
                                    rhs=w2_sb[:rows, fc, ob * 512:(ob + 1) * 512],
                                    start=(fc == 0), stop=(fc == NFFC - 1))
                            ye = y2e.tile([128, 512], F32, tag="ye")
                            nc.vector.tensor_tensor(out=ye, in0=ps,
                                                    in1=b2_t[:, ob * 512:(ob + 1) * 512],
                                                    op=mybir.AluOpType.add)
                            nc.sync.dma_start(
                                y2b[rr * 384 + o * 128: rr * 384 + (o + 1) * 128,
                                    ob * 512:(ob + 1) * 512], ye)

                nc.gpsimd.collective_compute(
                    "ReduceScatter", mybir.AluOpType.add,
                    replica_groups=[list(range(NCORES))],
                    ins=[y2_in[:].opt()], outs=[y2_out[:].opt()])

                y2r = y2_out[:].rearrange("(a b) -> a b", a=384)
                for b in range(NB):
                    yb = scr.tile([128, D], F32, tag="scr")
                    nc.sync.dma_start(yb, y2r.rearrange("(o p) f -> p o f", p=128)[:, b, :])
                    g_t = frame_tiles(b, VR_G_MLP)
                    t1 = scr.tile([128, D], F32, tag="scr")
                    nc.vector.tensor_tensor(out=t1, in0=yb, in1=g_t, op=mybir.AluOpType.mult)
                    nc.vector.tensor_tensor(out=x_sb[:, b, :], in0=x_sb[:, b, :], in1=t1,
                                            op=mybir.AluOpType.add)

            nc.sync.dma_start(out.ap().rearrange("(b p) f -> p b f", p=128), x_sb)

    nc.finalize()
    return nc


_CACHE = {}


def _prep_inputs(inputs):
    g = {k: np.asarray(v) for k, v in inputs.items()}
    x = np.ascontiguousarray(g["x"][0], np.float32)
    ctx = np.asarray(g["context"][0], np.float32)
    mod = (np.asarray(g["modulation"][0], np.float32)
           + np.asarray(g["t_mod"][0], np.float32))        # [F, 6, D]
    cos_f = np.asarray(g["freqs_cos"], np.float32)
    sin_f = np.asarray(g["freqs_sin"], np.float32)

    wq = np.ascontiguousarray(g["sa_wq"][:, ROPE_PERM]).astype(NPBF)
    wk = np.ascontiguousarray(g["sa_wk"][:, ROPE_PERM]).astype(NPBF)
    wv = np.asarray(g["sa_wv"]).astype(NPBF)
    wo = np.asarray(g["sa_wo"]).astype(NPBF)
    cq = np.asarray(g["ca_wq"]).astype(NPBF)
    ck = np.asarray(g["ca_wk"]).astype(NPBF)
    cv = np.asarray(g["ca_wv"]).astype(NPBF)
    cw = np.asarray(g["ca_wo"]).astype(NPBF)

    vecs = np.zeros((51, D), np.float32)
    vecs[VR_SH_MSA:VR_SH_MSA + 6] = mod[:, 0]
    vecs[VR_SC_MSA:VR_SC_MSA + 6] = 1.0 + mod[:, 1]
    vecs[VR_G_MSA:VR_G_MSA + 6] = mod[:, 2]
    vecs[VR_SH_MLP:VR_SH_MLP + 6] = mod[:, 3]
    vecs[VR_SC_MLP:VR_SC_MLP + 6] = 1.0 + mod[:, 4]
    vecs[VR_G_MLP:VR_G_MLP + 6] = mod[:, 5]
    vecs[VR_N3W] = g["norm3_w"]; vecs[VR_N3B] = g["norm3_b"]
    vecs[VR_NQ] = g["sa_nq"][ROPE_PERM]; vecs[VR_NK] = g["sa_nk"][ROPE_PERM]
    vecs[VR_CNQ] = g["ca_nq"]; vecs[VR_CNK] = g["ca_nk"]

    vecs[VR_BQ] = g["sa_bq"][ROPE_PERM]; vecs[VR_BK] = g["sa_bk"][ROPE_PERM]
    vecs[VR_BV] = g["sa_bv"]; vecs[VR_BO] = g["sa_bo"]
    vecs[VR_CBQ] = g["ca_bq"]; vecs[VR_CBK] = g["ca_bk"]
    vecs[VR_CBV] = g["ca_bv"]; vecs[VR_CBO] = g["ca_bo"]
    vecs[VR_B2] = g["ffn_b2"] / NCORES

    in_maps = []
    for r in range(NCORES):
        rows = _rows_of_core(r)
        w2p_ = np.zeros((FFP, D), np.float32)
        w2p_[:FFS] = g["ffn_w2"][r * FFS:(r + 1) * FFS]
        b1p_ = np.zeros((FFP,), np.float32)
        b1p_[:FFS] = g["ffn_b1"][r * FFS:(r + 1) * FFS]
        in_maps.append({
            "xs": np.ascontiguousarray(x[rows]),
            "cs": np.ascontiguousarray(
                np.concatenate([cos_f[rows], sin_f[rows]], axis=1), dtype=np.float32),
            "wq": wq, "wk": wk, "wv": wv, "wo": wo,
            "cq": cq, "ck": ck, "cv": cv, "cw": cw,
            "ctxT": np.ascontiguousarray(ctx[r * 64:(r + 1) * 64].T).astype(NPBF),
            "w1": np.ascontiguousarray(
                g["ffn_w1"][:, r * FFS:(r + 1) * FFS]).astype(NPBF),
            "w2": w2p_.astype(NPBF),
            "b1": np.ascontiguousarray(b1p_.reshape(NFFC, 128).T),
            "vecs": vecs,
        })
    return in_maps


def _run(inputs, trace=False):
    if "nc" not in _CACHE:
        _CACHE["nc"] = _build()
    nc = _CACHE["nc"]
    in_maps = _prep_inputs(inputs)
    res = run_bass_kernel_spmd(nc, in_maps, core_ids=list(range(NCORES)), trace=trace)
    out = np.empty((1, T, D), np.float32)
    for r in range(NCORES):
        out[0, _rows_of_core(r)] = res.results[r]["y"]
    return out, res


def kernel(**inputs):
    out, _ = _run(inputs, trace=False)
    return out


# revision 9
# speedup vs baseline: 1.2196x; 1.2196x over previous
"""CausalDiT block on 8 TRN2 NeuronCores.

Sharding: frame-strided sequence shard — core r owns rows [512f+64r, 512f+64r+64)
for every frame f (perfect causal balance, SPMD-uniform suffix structure).
Self/cross-attention QKV+O computed on own rows with full (bf16) weights;
K/V (pre-transposed K) AllGathered; FFN is tensor-parallel on the hidden dim
with AllGather(inp^T) + ReduceScatter(partial y).

All matmuls run in bf16 (weights cast host-side), accumulation fp32 in PSUM;
norms/softmax/residual chains in fp32. RoPE is applied split-half after a
host-side even/odd permutation of wq/wk output columns (dot-product invariant).
"""
import numpy as np
import ml_dtypes

import concourse.bass as bass
import concourse.mybir as mybir
import concourse.tile as tile
from concourse import bacc
from concourse.bass_utils import run_bass_kernel_spmd
from concourse.masks import make_identity

BF = mybir.dt.bfloat16
F32 = mybir.dt.float32
NPBF = ml_dtypes.bfloat16

NCORES = 8
T, D, H, HD, F, S, FF, TC = 3072, 1536, 12, 128, 6, 512, 8960, 512
FFS = FF // NCORES            # 1120 hidden per core
FFP = 1152                    # padded to 9*128
NFFC = 9                      # ff chunks (8 full + one 96)
NB = 3                        # local 128-row blocks (384 rows/core)
NCH = D // 128                # 12 chunks of the model dim
EPS = 1e-6
SCALE = float(1.0 / np.sqrt(HD))
KVSEG = D * 384               # elements of one packed AG segment (kT or v)
CASEG = D * 64

# vecs rows
VR_SH_MSA, VR_SC_MSA, VR_G_MSA = 0, 6, 12
VR_SH_MLP, VR_SC_MLP, VR_G_MLP = 18, 24, 30
VR_N3W, VR_N3B, VR_NQ, VR_NK, VR_CNQ, VR_CNK = 36, 37, 38, 39, 40, 41
VR_BQ, VR_BK, VR_BV, VR_BO, VR_CBQ, VR_CBK, VR_CBV, VR_CBO, VR_B2 = range(42, 51)

_rope_perm_head = np.concatenate([np.arange(0, HD, 2), np.arange(1, HD, 2)])
ROPE_PERM = np.concatenate([h * HD + _rope_perm_head for h in range(H)])


def _rows_of_core(r):
    return np.concatenate([512 * f + 64 * r + np.arange(64) for f in range(F)])


def _bcast_rows(handle, row, nparts, ncols=D):
    """DRAM AP replicating `row` of a [R, ncols] tensor across nparts partitions."""
    return bass.AP(tensor=handle, offset=row * ncols, ap=[[0, nparts], [1, ncols]])


def _build():
    nc = bacc.Bacc("TRN2", target_bir_lowering=False, debug=False, num_devices=NCORES)

    di = {}
    def inp(name, shape, dt):
        di[name] = nc.dram_tensor(name, shape, dt, kind="ExternalInput")
        return di[name]

    xs = inp("xs", [384, D], F32)
    cs = inp("cs", [384, 128], F32)            # [cos | sin] per own row
    for w in ["wq", "wk", "wv", "wo", "cq", "ck", "cv", "cw"]:
        inp(w, [D, D], BF)
    ctxT = inp("ctxT", [D, 64], BF)
    w1 = inp("w1", [D, FFS], BF)
    w2 = inp("w2", [FFP, D], BF)
    b1 = inp("b1", [128, NFFC], F32)
    vecs = inp("vecs", [51, D], F32)
    out = nc.dram_tensor("y", [384, D], F32, kind="ExternalOutput")

    def wview(name):
        return di[name].ap().rearrange("(o p) f -> p o f", p=128)

    with tile.TileContext(nc) as tc:
        import contextlib
        ctx = contextlib.ExitStack()
        with ctx:
            # ---- persistent pools ----
            dram = ctx.enter_context(tc.tile_pool(name="dram", bufs=1, space="DRAM"))
            singles = ctx.enter_context(tc.tile_pool(name="singles", bufs=1))
            xp = ctx.enter_context(tc.tile_pool(name="xp", bufs=1))
            scr = ctx.enter_context(tc.tile_pool(name="scr", bufs=2))
            statp = ctx.enter_context(tc.tile_pool(name="statp", bufs=2))
            replp = ctx.enter_context(tc.tile_pool(name="replp", bufs=2))
            actbf = ctx.enter_context(tc.tile_pool(name="actbf", bufs=2))
            tpTp = ctx.enter_context(tc.tile_pool(name="tpTp", bufs=1))
            qTp = ctx.enter_context(tc.tile_pool(name="qTp", bufs=1))
            aoTp = ctx.enter_context(tc.tile_pool(name="aoTp", bufs=1))
            pbfp = ctx.enter_context(tc.tile_pool(name="pbfp", bufs=2))
            rrep = ctx.enter_context(tc.tile_pool(name="rrep", bufs=2))
            # psum pools
            ps_pp = ctx.enter_context(tc.tile_pool(name="ps_pp", bufs=2, space="PSUM"))
            ps_sp = ctx.enter_context(tc.tile_pool(name="ps_sp", bufs=2, space="PSUM"))
            ps_op = ctx.enter_context(tc.tile_pool(name="ps_op", bufs=2, space="PSUM"))
            ps_mp = ctx.enter_context(tc.tile_pool(name="ps_mp", bufs=1, space="PSUM"))
            ps_tp = ctx.enter_context(tc.tile_pool(name="ps_tp", bufs=1, space="PSUM"))

            # ---- constants ----
            idon = singles.tile([128, 256], BF)          # [:,0:128] identity, [:,128:256] ones
            make_identity(nc, idon[:, 0:128])
            nc.vector.memset(idon[:, 128:256], 1.0)
            ones_mat = idon[:, 128:256]                  # [128,128] of ones
            cs_sb = singles.tile([128, NB, 128], F32)    # [p, b, cos|sin]
            nc.sync.dma_start(cs_sb, cs.ap().rearrange("(b p) d -> p b d", p=128))
            b1_sb = singles.tile([128, NFFC], F32)
            nc.sync.dma_start(b1_sb, b1.ap())
            eps_sb = singles.tile([128, 1], F32)
            nc.vector.memset(eps_sb, EPS)
            x_sb = xp.tile([128, NB, D], F32)
            nc.sync.dma_start(x_sb, xs.ap().rearrange("(b p) f -> p b f", p=128))

            # ---- DRAM bounce buffers ----
            kv_in = dram.tile([2 * KVSEG], BF)
            kv_out = dram.tile([NCORES * 2 * KVSEG], BF, addr_space="Shared")
            ca_in = dram.tile([2 * CASEG], BF)
            ca_out = dram.tile([NCORES * 2 * CASEG], BF, addr_space="Shared")
            i2_in = dram.tile([KVSEG], BF)
            i2_out = dram.tile([NCORES * KVSEG], BF, addr_space="Shared")
            y2_in = dram.tile([T * D], BF)
            y2_out = dram.tile([384 * D], BF)

            # ================= helpers =================
            def repl(rows64=None, row=None, nparts=128):
                """Replicated [128, D] f32 tile from vecs rows (DMA broadcast).
                rows64: list of (part_off, nparts, row) half-fills."""
                t = replp.tile([128, D], F32, tag="repl")
                if rows64 is not None:
                    for off, n, rw in rows64:
                        nc.sync.dma_start(t[off:off + n, :], _bcast_rows(vecs, rw, n))
                else:
                    nc.sync.dma_start(t[:nparts, :], _bcast_rows(vecs, row, nparts))
                return t

            def frame_tiles(b, base_row):
                """(1+sc)/sh/g style tile for block b: frames 2b (rows 0-63), 2b+1."""
                return repl(rows64=[(0, 64, base_row + 2 * b), (64, 64, base_row + 2 * b + 1)])

            def layer_norm(src_ap, c_tile, s_tile, out_bf):
                """out_bf (bf16 [128,D]) = LN(src) * c_tile + s_tile."""
                st = statp.tile([128, 24], F32, tag="stat")
                sv = src_ap.rearrange("p (n d) -> p n d", d=512)
                for i in range(3):
                    nc.vector.bn_stats(st[:, 6 * i:6 * i + 6], sv[:, i, :])
                nc.vector.bn_aggr(st[:, 18:20], st[:, 0:18].rearrange("p (n d) -> p n d", d=6))
                nc.scalar.activation(st[:, 20:21], st[:, 19:20],
                                     mybir.ActivationFunctionType.Sqrt, bias=eps_sb)
                nc.vector.reciprocal(st[:, 20:21], st[:, 20:21])
                t1 = scr.tile([128, D], F32, tag="scr")
                nc.vector.tensor_scalar(out=t1, in0=src_ap,
                                        scalar1=st[:, 18:19], scalar2=st[:, 20:21],
                                        op0=mybir.AluOpType.subtract,
                                        op1=mybir.AluOpType.mult)
                t2 = scr.tile([128, D], F32, tag="scr")
                nc.vector.tensor_tensor(out=t2, in0=t1, in1=c_tile, op=mybir.AluOpType.mult)
                nc.vector.tensor_tensor(out=out_bf, in0=t2, in1=s_tile, op=mybir.AluOpType.add)

            def transpose_into(src_bf, dst, coff, nch=NCH, rows=128):
                """PE-transpose src_bf [rows, nch*128] into dst[:, c, coff:coff+rows]."""
                for c in range(nch):
                    pt = ps_tp.tile([128, 128], BF, tag="tr")
                    nc.tensor.transpose(pt[:, :rows], src_bf[:rows, c * 128:(c + 1) * 128],
                                        idon[:rows, 0:rows])
                    nc.vector.tensor_copy(out=dst[:, c, coff:coff + rows], in_=pt[:, :rows])

            def project(wname, brow, lhsT_sb, evict, m=128, nb=NB):
                """psum[b] = lhsT[b].T @ W ; evict(b, ob, psum[:m,:512], bias_tile)."""
                wv = wview(wname)
                bias_t = repl(row=brow, nparts=m)
                for ob in range(3):
                    wt = bigw.tile([128, NCH, 512], BF, tag="bigw")
                    nc.sync.dma_start(wt, wv[:, :, ob * 512:(ob + 1) * 512])
                    for b in range(nb):
                        ps = ps_pp.tile([128, 512], F32, tag="pp")
                        for c in range(NCH):
                            nc.tensor.matmul(ps[:m, :], lhsT=lhsT_sb[:, c, b * 128:b * 128 + m],
                                             rhs=wt[:, c, :], start=(c == 0),
                                             stop=(c == NCH - 1))
                        evict(b, ob, ps, bias_t)

            def rms_norm_rope(qf_tiles, nrow, dst_T, rope):
                """qf_tiles: 3 blocks [128,D] f32 -> rms(nq)+optional rope -> bf16 -> dst_T."""
                nrm = repl(row=nrow)
                for b in range(NB):
                    st = statp.tile([128, 24], F32, tag="stat")
                    sq = scr.tile([128, D], F32, tag="scr")
                    nc.scalar.activation(sq, qf_tiles[b], mybir.ActivationFunctionType.Square,
                                         accum_out=st[:, 0:1])
                    nc.scalar.activation(st[:, 1:2], st[:, 0:1],
                                         mybir.ActivationFunctionType.Sqrt,
                                         bias=eps_sb, scale=1.0 / D)
                    nc.vector.reciprocal(st[:, 1:2], st[:, 1:2])
                    qbf = actbf.tile([128, D], BF, tag="abf")
                    if not rope:
                        nc.vector.scalar_tensor_tensor(out=qbf, in0=qf_tiles[b],
                                                       scalar=st[:, 1:2], in1=nrm,
                                                       op0=mybir.AluOpType.mult,
                                                       op1=mybir.AluOpType.mult)
                    else:
                        qn = scr.tile([128, D], F32, tag="scr")
                        nc.vector.scalar_tensor_tensor(out=qn, in0=qf_tiles[b],
                                                       scalar=st[:, 1:2], in1=nrm,
                                                       op0=mybir.AluOpType.mult,
                                                       op1=mybir.AluOpType.mult)
                        qv = qn.rearrange("p (h two d) -> p h two d", two=2, d=64)
                        ov = qbf.rearrange("p (h two d) -> p h two d", two=2, d=64)
                        cb = cs_sb[:, b:b + 1, 0:64].to_broadcast((128, H, 64))
                        sb_ = cs_sb[:, b:b + 1, 64:128].to_broadcast((128, H, 64))
                        tt = scr.tile([128, D], F32, tag="scr")
                        t1 = tt[:, 0:768].rearrange("p (h d) -> p h d", d=64)
                        t2 = tt[:, 768:1536].rearrange("p (h d) -> p h d", d=64)
                        top, bot = qv[:, :, 0, :], qv[:, :, 1, :]
                        nc.vector.tensor_tensor(out=t1, in0=top, in1=cb, op=mybir.AluOpType.mult)
                        nc.vector.tensor_tensor(out=t2, in0=bot, in1=sb_, op=mybir.AluOpType.mult)
                        nc.vector.tensor_tensor(out=ov[:, :, 0, :], in0=t1, in1=t2,
                                                op=mybir.AluOpType.subtract)
                        nc.vector.tensor_tensor(out=t1, in0=top, in1=sb_, op=mybir.AluOpType.mult)
                        nc.vector.tensor_tensor(out=t2, in0=bot, in1=cb, op=mybir.AluOpType.mult)
                        nc.vector.tensor_tensor(out=ov[:, :, 1, :], in0=t1, in1=t2,
                                                op=mybir.AluOpType.add)
                    transpose_into(qbf, dst_T, b * 128)

            def attention(qT, kt_of, vh_of, kseq, dst_oT):
                """Generic S^T attention.
                kt_of(h) -> [128, nkt*128] lhsT source tile (head dims x keys per tile)
                vh_of(h) -> [128, nkt*128]-style v tile accessor
                kseq: list of (ktile_idx, qoff) in processing order."""
                for h in range(H):
                    kt, vh = kt_of(h), vh_of(h)
                    ps_o = ps_op.tile([128, 384], F32, tag="po")
                    ps_m = ps_mp.tile([128, 384], F32, tag="pm")
                    for i, (ki, qo) in enumerate(kseq):
                        n = 384 - qo
                        ps_s = ps_sp.tile([128, 384], F32, tag="ps")
                        nc.tensor.matmul(ps_s[:, 0:n], lhsT=kt[:, ki * 128:(ki + 1) * 128],
                                         rhs=qT[:, h, qo:384], start=True, stop=True)
                        pbt = pbfp.tile([128, 384], BF, tag="pbf")
                        nc.scalar.activation(pbt[:, 0:n], ps_s[:, 0:n],
                                             mybir.ActivationFunctionType.Exp, scale=SCALE)
                        last = i == len(kseq) - 1
                        nc.tensor.matmul(ps_o[:, qo:384], lhsT=vh[:, ki * 128:(ki + 1) * 128],
                                         rhs=pbt[:, 0:n], start=(i == 0), stop=last,
                                         skip_group_check=True)
                        nc.tensor.matmul(ps_m[:, qo:384], lhsT=ones_mat,
                                         rhs=pbt[:, 0:n], start=(i == 0), stop=last,
                                         skip_group_check=True)
                    rc = rrep.tile([128, 384], F32, tag="rr")
                    nc.vector.reciprocal(rc, ps_m)
                    nc.vector.tensor_tensor(out=dst_oT[:, h, :], in0=ps_o, in1=rc,
                                            op=mybir.AluOpType.mult)

            # ============================================================
            # Phase 1+2: self-attention & cross-attention
            # ============================================================
            with tc.tile_pool(name="bigw", bufs=2) as bigw, \
                 tc.tile_pool(name="qfp", bufs=3) as qfp, \
                 tc.tile_pool(name="vsb", bufs=1) as vsbp, \
                 tc.tile_pool(name="midp", bufs=2) as midp:

                # ---- LN1 + modulation -> inp (bf16, transposed) ----
                inpT = tpTp.tile([128, NCH, 384], BF, tag="tpT")
                for b in range(NB):
                    c_t = frame_tiles(b, VR_SC_MSA)
                    s_t = frame_tiles(b, VR_SH_MSA)
                    ibf = actbf.tile([128, D], BF, tag="abf")
                    layer_norm(x_sb[:, b, :], c_t, s_t, ibf)
                    transpose_into(ibf, inpT, b * 128)

                # ---- QKV projections ----
                def ev_f32(tiles):
                    def e(b, ob, ps, bias_t):
                        sl = slice(ob * 512, (ob + 1) * 512)
                        nc.vector.tensor_tensor(out=tiles[b][:, sl], in0=ps,
                                                in1=bias_t[:, sl], op=mybir.AluOpType.add)
                    return e
                kf = [qfp.tile([128, D], F32, tag="qf", name=f"kf{i}") for i in range(NB)]
                project("wk", VR_BK, inpT, ev_f32(kf))
                kT_l = tpTp.tile([128, NCH, 384], BF, tag="kTl")
                rms_norm_rope(kf, VR_NK, kT_l, rope=True)

                v_sb = vsbp.tile([128, NB, D], BF, tag="vsb")
                def ev_v(b, ob, ps, bias_t):
                    sl = slice(ob * 512, (ob + 1) * 512)
                    nc.vector.tensor_tensor(out=v_sb[:, b, sl], in0=ps,
                                            in1=bias_t[:, sl], op=mybir.AluOpType.add)
                project("wv", VR_BV, inpT, ev_v)

                # ---- pack + AllGather kT|v ----
                kT_seg = kv_in[0:KVSEG].rearrange("(a b) -> a b", a=D)
                v_seg = kv_in[KVSEG:2 * KVSEG].rearrange("(a b) -> a b", a=384)
                nc.sync.dma_start(kT_seg.rearrange("(o p) f -> p o f", p=128), kT_l)
                nc.sync.dma_start(v_seg.rearrange("(o p) f -> p o f", p=128), v_sb)
                nc.gpsimd.collective_compute(
                    "AllGather", mybir.AluOpType.bypass,
                    replica_groups=[list(range(NCORES))],
                    ins=[kv_in[:].opt()], outs=[kv_out[:].opt()])

                # context k/v on own 64 context rows
                ctxT_sb = singles.tile([128, NCH, 64], BF)
                nc.sync.dma_start(ctxT_sb, ctxT.ap().rearrange("(o p) f -> p o f", p=128))
                kc_f = qfp.tile([128, D], F32, tag="qf")
                def ev_kc(b, ob, ps, bias_t):
                    sl = slice(ob * 512, (ob + 1) * 512)
                    nc.vector.tensor_tensor(out=kc_f[0:64, sl], in0=ps[:64, :],
                                            in1=bias_t[:64, sl], op=mybir.AluOpType.add)
                project("ck", VR_CBK, ctxT_sb, ev_kc, m=64, nb=1)
                vc_bf = actbf.tile([128, D], BF, tag="abf")
                def ev_vc(b, ob, ps, bias_t):
                    sl = slice(ob * 512, (ob + 1) * 512)
                    nc.vector.tensor_tensor(out=vc_bf[0:64, sl], in0=ps[:64, :],
                                            in1=bias_t[:64, sl], op=mybir.AluOpType.add)
                project("cv", VR_CBV, ctxT_sb, ev_vc, m=64, nb=1)

                # rms norm kc (64 rows)
                cnk = repl(row=VR_CNK, nparts=64)
                stc = statp.tile([128, 24], F32, tag="stat")
                sqc = scr.tile([128, D], F32, tag="scr")
                nc.scalar.activation(sqc[:64, :], kc_f[:64, :],
                                     mybir.ActivationFunctionType.Square,
                                     accum_out=stc[:64, 0:1])
                nc.scalar.activation(stc[:64, 1:2], stc[:64, 0:1],
                                     mybir.ActivationFunctionType.Sqrt, bias=eps_sb[:64],
                                     scale=1.0 / D)
                nc.vector.reciprocal(stc[:64, 1:2], stc[:64, 1:2])
                kc_bf = actbf.tile([128, D], BF, tag="abf")
                nc.vector.scalar_tensor_tensor(out=kc_bf[:64, :], in0=kc_f[:64, :],
                                               scalar=stc[:64, 1:2], in1=cnk[:64, :],
                                               op0=mybir.AluOpType.mult,
                                               op1=mybir.AluOpType.mult)
                kcT_l = tpTp.tile([128, NCH, 64], BF, tag="kcTl")
                transpose_into(kc_bf, kcT_l, 0, rows=64)

                # pack + AllGather ca kT|v
                kcT_seg = ca_in[0:CASEG].rearrange("(a b) -> a b", a=D)
                vc_seg = ca_in[CASEG:2 * CASEG].rearrange("(a b) -> a b", a=64)
                nc.sync.dma_start(kcT_seg.rearrange("(o p) f -> p o f", p=128), kcT_l)
                nc.sync.dma_start(vc_seg, vc_bf[:64, :])
                nc.gpsimd.collective_compute(
                    "AllGather", mybir.AluOpType.bypass,
                    replica_groups=[list(range(NCORES))],
                    ins=[ca_in[:].opt()], outs=[ca_out[:].opt()])

                qf = [qfp.tile([128, D], F32, tag="qf", name=f"qf{i}") for i in range(NB)]
                qT = qTp.tile([128, NCH, 384], BF, tag="qT")
                project("wq", VR_BQ, inpT, ev_f32(qf))
                rms_norm_rope(qf, VR_NQ, qT, rope=True)

                # ---- SA attention ----
                SA_SEQ = [(f * 4 + j, 64 * f) for f in range(F) for j in range(4)]

                def sa_kt2(h):
                    kt = midp.tile([128, F * 512], BF, tag="midk")
                    ktv = kt.rearrange("p (f d) -> p f d", d=512)
                    for rr in range(NCORES):
                        seg = kv_out[rr * 2 * KVSEG: rr * 2 * KVSEG + KVSEG] \
                            .rearrange("(a b) -> a b", a=D)
                        nc.sync.dma_start(
                            ktv[:, :, rr * 64:(rr + 1) * 64],
                            seg[h * 128:(h + 1) * 128, :].rearrange("p (f d) -> p f d", d=64))
                    return kt

                def sa_vh(h):
                    vh = midp.tile([128, F * 512], BF, tag="midv")
                    vhv = vh.rearrange("p (f d) -> p f d", d=512)
                    for rr in range(NCORES):
                        seg = kv_out[rr * 2 * KVSEG + KVSEG: (rr + 1) * 2 * KVSEG] \
                            .rearrange("(a b) -> a b", a=384)
                        nc.sync.dma_start(
                            vhv[64 * (rr % 2):64 * (rr % 2) + 64, :,
                                (rr // 2) * 128:(rr // 2) * 128 + 128],
                            seg.rearrange("(f p) c -> p f c", p=64)[:, :, h * 128:(h + 1) * 128])
                    return vh

                attn_oT = aoTp.tile([128, NCH, 384], BF, tag="aoT")
                attention(qT, sa_kt2, sa_vh, SA_SEQ, attn_oT)

                # ---- output projection + gated residual ----
                ysa = [qfp.tile([128, D], F32, tag="qf", name=f"ysa{i}") for i in range(NB)]
                project("wo", VR_BO, attn_oT, ev_f32(ysa))
                for b in range(NB):
                    g_t = frame_tiles(b, VR_G_MSA)
                    t1 = scr.tile([128, D], F32, tag="scr")
                    nc.vector.tensor_tensor(out=t1, in0=ysa[b], in1=g_t, op=mybir.AluOpType.mult)
                    nc.vector.tensor_tensor(out=x_sb[:, b, :], in0=x_sb[:, b, :], in1=t1,
                                            op=mybir.AluOpType.add)

                # ============ cross attention ============
                n3w = repl(row=VR_N3W)
                n3b = repl(row=VR_N3B)
                xnT = tpTp.tile([128, NCH, 384], BF, tag="tpT")
                for b in range(NB):
                    xbf = actbf.tile([128, D], BF, tag="abf")
                    layer_norm(x_sb[:, b, :], n3w, n3b, xbf)
                    transpose_into(xbf, xnT, b * 128)

                qcf = [qfp.tile([128, D], F32, tag="qf", name=f"qcf{i}") for i in range(NB)]
                project("cq", VR_CBQ, xnT, ev_f32(qcf))
                qcT = qTp.tile([128, NCH, 384], BF, tag="qT")
                rms_norm_rope(qcf, VR_CNQ, qcT, rope=False)

                CA_SEQ = [(t, 0) for t in range(4)]

                def ca_kt(h):
                    kt = midp.tile([128, F * 512], BF, tag="midk")
                    for rr in range(NCORES):
                        seg = ca_out[rr * 2 * CASEG: rr * 2 * CASEG + CASEG] \
                            .rearrange("(a b) -> a b", a=D)
                        nc.sync.dma_start(kt[:, rr * 64:(rr + 1) * 64],
                                          seg[h * 128:(h + 1) * 128, :])
                    return kt

                def ca_vh(h):
                    vh = midp.tile([128, F * 512], BF, tag="midv")
                    for rr in range(NCORES):
                        seg = ca_out[rr * 2 * CASEG + CASEG: (rr + 1) * 2 * CASEG] \
                            .rearrange("(a b) -> a b", a=64)
                        nc.sync.dma_start(
                            vh[64 * (rr % 2):64 * (rr % 2) + 64,
                               (rr // 2) * 128:(rr // 2) * 128 + 128],
                            seg[:, h * 128:(h + 1) * 128])
                    return vh

                attn2_oT = aoTp.tile([128, NCH, 384], BF, tag="aoT")
                attention(qcT, ca_kt, ca_vh, CA_SEQ, attn2_oT)

                # output projection, residual (no gate)
                def ev_addx(b, ob, ps, bias_t):
                    sl = slice(ob * 512, (ob + 1) * 512)
                    t9 = scr.tile([128, 512], F32, tag="scr512")
                    nc.vector.tensor_tensor(out=t9, in0=ps, in1=bias_t[:, sl],
                                            op=mybir.AluOpType.add)
                    nc.vector.tensor_tensor(out=x_sb[:, b, sl], in0=x_sb[:, b, sl],
                                            in1=t9, op=mybir.AluOpType.add)
                project("cw", VR_CBO, attn2_oT, ev_addx)

            # ============================================================
            # Phase 3: FFN (tensor-parallel on hidden dim)
            # ============================================================
            with tc.tile_pool(name="w1p", bufs=1) as w1p, \
                 tc.tile_pool(name="w2p", bufs=1) as w2p, \
                 tc.tile_pool(name="i2p", bufs=2) as i2p, \
                 tc.tile_pool(name="hTp", bufs=2) as hTp, \
                 tc.tile_pool(name="y2e", bufs=2) as y2e:

                inp2T = tpTp.tile([128, NCH, 384], BF, tag="tpT")
                for b in range(NB):
                    c_t = frame_tiles(b, VR_SC_MLP)
                    s_t = frame_tiles(b, VR_SH_MLP)
                    ibf = actbf.tile([128, D], BF, tag="abf")
                    layer_norm(x_sb[:, b, :], c_t, s_t, ibf)
                    transpose_into(ibf, inp2T, b * 128)
                i2_seg = i2_in[:].rearrange("(a b) -> a b", a=D)
                nc.sync.dma_start(i2_seg.rearrange("(o p) f -> p o f", p=128), inp2T)
                nc.gpsimd.collective_compute(
                    "AllGather", mybir.AluOpType.bypass,
                    replica_groups=[list(range(NCORES))],
                    ins=[i2_in[:].opt()], outs=[i2_out[:].opt()])

                w1_sb = w1p.tile([128, NCH, FFS], BF, tag="w1")
                nc.sync.dma_start(w1_sb, w1.ap().rearrange("(o p) f -> p o f", p=128))
                w2_sb = w2p.tile([128, NFFC, D], BF, tag="w2")
                b2_t = repl(row=VR_B2)
                nc.sync.dma_start(w2_sb, w2.ap().rearrange("(o p) f -> p o f", p=128))
                y2b = y2_in[:].rearrange("(a b) -> a b", a=T)

                for rr in range(NCORES):
                    i2t = i2p.tile([128, NCH, 384], BF, tag="i2")
                    seg = i2_out[rr * KVSEG:(rr + 1) * KVSEG].rearrange("(a b) -> a b", a=D)
                    nc.sync.dma_start(i2t, seg.rearrange("(o p) f -> p o f", p=128))
                    hT = hTp.tile([128, NFFC, 384], BF, tag="hT")
                    for fc in range(NFFC):
                        rows = 96 if fc == NFFC - 1 else 128
                        ps = ps_pp.tile([128, 512], F32, tag="pp")
                        for c in range(NCH):
                            nc.tensor.matmul(ps[:rows, 0:384],
                                             lhsT=w1_sb[:, c, fc * 128:fc * 128 + rows],
                                             rhs=i2t[:, c, :], start=(c == 0),
                                             stop=(c == NCH - 1))
                        nc.scalar.activation(hT[:rows, fc, :], ps[:rows, 0:384],
                                             mybir.ActivationFunctionType.Gelu_apprx_tanh,
                                             bias=b1_sb[:rows, fc:fc + 1])
                    for o in range(NB):
                        for ob in range(3):
                            ps = ps_pp.tile([128, 512], F32, tag="pp")
                            for fc in range(NFFC):
                                rows = 96 if fc == NFFC - 1 else 128
                                nc.tensor.matmul(
                                    ps, lhsT=hT[:rows, fc, o * 128:(o + 1) * 128],
                                    rhs=w2_sb[:rows, fc, ob * 512:(ob + 1) * 512],
                                    start=(fc == 0), stop=(fc == NFFC - 1))
                            ye = y2e.tile([128, 512], BF, tag="ye")
                            nc.vector.tensor_tensor(out=ye, in0=ps,
                                                    in1=b2_t[:, ob * 512:(ob + 1) * 512],
                                                    op=mybir.AluOpType.add)
                            nc.sync.dma_start(
                                y2b[rr * 384 + o * 128: rr * 384 + (o + 1) * 128,
                                    ob * 512:(ob + 1) * 512], ye)

                nc.gpsimd.collective_compute(
                    "ReduceScatter", mybir.AluOpType.add,
                    replica_groups=[list(range(NCORES))],
                    ins=[y2_in[:].opt()], outs=[y2_out[:].opt()])

                y2r = y2_out[:].rearrange("(a b) -> a b", a=384)
                for b in range(NB):
                    yb = scr.tile([128, D], BF, tag="ybbf")
                    nc.sync.dma_start(yb, y2r.rearrange("(o p) f -> p o f", p=128)[:, b, :])
                    g_t = frame_tiles(b, VR_G_MLP)
                    t1 = scr.tile([128, D], F32, tag="scr")
                    nc.vector.tensor_tensor(out=t1, in0=yb, in1=g_t, op=mybir.AluOpType.mult)
                    nc.vector.tensor_tensor(out=x_sb[:, b, :], in0=x_sb[:, b, :], in1=t1,
                                            op=mybir.AluOpType.add)

            nc.sync.dma_start(out.ap().rearrange("(b p) f -> p b f", p=128), x_sb)

    nc.finalize()
    return nc


_CACHE = {}


def _prep_inputs(inputs):
    g = {k: np.asarray(v) for k, v in inputs.items()}
    x = np.ascontiguousarray(g["x"][0], np.float32)
    ctx = np.asarray(g["context"][0], np.float32)
    mod = (np.asarray(g["modulation"][0], np.float32)
           + np.asarray(g["t_mod"][0], np.float32))        # [F, 6, D]
    cos_f = np.asarray(g["freqs_cos"], np.float32)
    sin_f = np.asarray(g["freqs_sin"], np.float32)

    wq = np.ascontiguousarray(g["sa_wq"][:, ROPE_PERM]).astype(NPBF)
    wk = np.ascontiguousarray(g["sa_wk"][:, ROPE_PERM]).astype(NPBF)
    wv = np.asarray(g["sa_wv"]).astype(NPBF)
    wo = np.asarray(g["sa_wo"]).astype(NPBF)
    cq = np.asarray(g["ca_wq"]).astype(NPBF)
    ck = np.asarray(g["ca_wk"]).astype(NPBF)
    cv = np.asarray(g["ca_wv"]).astype(NPBF)
    cw = np.asarray(g["ca_wo"]).astype(NPBF)

    vecs = np.zeros((51, D), np.float32)
    vecs[VR_SH_MSA:VR_SH_MSA + 6] = mod[:, 0]
    vecs[VR_SC_MSA:VR_SC_MSA + 6] = 1.0 + mod[:, 1]
    vecs[VR_G_MSA:VR_G_MSA + 6] = mod[:, 2]
    vecs[VR_SH_MLP:VR_SH_MLP + 6] = mod[:, 3]
    vecs[VR_SC_MLP:VR_SC_MLP + 6] = 1.0 + mod[:, 4]
    vecs[VR_G_MLP:VR_G_MLP + 6] = mod[:, 5]
    vecs[VR_N3W] = g["norm3_w"]; vecs[VR_N3B] = g["norm3_b"]
    vecs[VR_NQ] = g["sa_nq"][ROPE_PERM]; vecs[VR_NK] = g["sa_nk"][ROPE_PERM]
    vecs[VR_CNQ] = g["ca_nq"]; vecs[VR_CNK] = g["ca_nk"]

    vecs[VR_BQ] = g["sa_bq"][ROPE_PERM]; vecs[VR_BK] = g["sa_bk"][ROPE_PERM]
    vecs[VR_BV] = g["sa_bv"]; vecs[VR_BO] = g["sa_bo"]
    vecs[VR_CBQ] = g["ca_bq"]; vecs[VR_CBK] = g["ca_bk"]
    vecs[VR_CBV] = g["ca_bv"]; vecs[VR_CBO] = g["ca_bo"]
    vecs[VR_B2] = g["ffn_b2"] / NCORES

    in_maps = []
    for r in range(NCORES):
        rows = _rows_of_core(r)
        w2p_ = np.zeros((FFP, D), np.float32)
        w2p_[:FFS] = g["ffn_w2"][r * FFS:(r + 1) * FFS]
        b1p_ = np.zeros((FFP,), np.float32)
        b1p_[:FFS] = g["ffn_b1"][r * FFS:(r + 1) * FFS]
        in_maps.append({
            "xs": np.ascontiguousarray(x[rows]),
            "cs": np.ascontiguousarray(
                np.concatenate([cos_f[rows], sin_f[rows]], axis=1), dtype=np.float32),
            "wq": wq, "wk": wk, "wv": wv, "wo": wo,
            "cq": cq, "ck": ck, "cv": cv, "cw": cw,
            "ctxT": np.ascontiguousarray(ctx[r * 64:(r + 1) * 64].T).astype(NPBF),
            "w1": np.ascontiguousarray(
                g["ffn_w1"][:, r * FFS:(r + 1) * FFS]).astype(NPBF),
            "w2": w2p_.astype(NPBF),
            "b1": np.ascontiguousarray(b1p_.reshape(NFFC, 128).T),
            "vecs": vecs,
        })
    return in_maps


def _run(inputs, trace=False):
    if "nc" not in _CACHE:
        _CACHE["nc"] = _build()
    nc = _CACHE["nc"]
    in_maps = _prep_inputs(inputs)
    res = run_bass_kernel_spmd(nc, in_maps, core_ids=list(range(NCORES)), trace=trace)
    out = np.empty((1, T, D), np.float32)
    for r in range(NCORES):
        out[0, _rows_of_core(r)] = res.results[r]["y"]
    return out, res


def kernel(**inputs):
    out, _ = _run(inputs, trace=False)
    return out
